# revision 26
# baseline (speedup 1.0000x reference)
"""Trainium2 Bass kernel for nn_NSELoss (segment-reduce NSE loss).

Contract: kernel(**inputs) takes the FULL inputs
  y_pred [16777216] f32, y_true [16777216] f32,
  stations [16777216] i32, station_std [1024] f32
and returns the full scalar output (f32), matching reference().
y_pred/y_true are cast to bf16 and stations to int16 during host-side
sharding (cuts HBM traffic 24 -> 12 MB/core; the on-device math is bf16
anyway and station ids < 1024 are exact in i16 — adds ~1e-4-level noise
against the 2e-2 tolerance). The hi digit is kept unshifted (st & 960)
and compared against 64*a, since i16 shifts fail walrus's ISA check.

Sharding: data-parallel over N across 8 NeuronCores. Each core computes
partial sse[1024] and cnt[1024]; host sums the 8 partials and finishes.

Device algorithm (per core, 2,097,152 elements as [128, 16384]):
Split the station id s = hi*64 + lo (hi 4 bits, lo 6 bits). Per chunk of
G columns:
  DVE: lo/hi extraction (tensor_scalar 2x + copy), diff = yp - yt,
       A1[a]   = (hi == a), a in 0..6    tensor_scalar is_equal @ 4x mode
       V16[16] = A1 * err2               ONE fused tensor_tensor (bcast
                                         on the outer AP dim) @ 2x mode
       B[64]   = (lo == j)               64x tensor_scalar is_equal @ 4x
  ACT: err2 = Square(diff) -> bf16, and A1[a] for a in 7..15 via the
       exact integer one-hot relu(1 - (hi - a)^2)  (2 passes/class) —
       offloads the Vector engine, which is otherwise the bottleneck.
  PE:  per column c, one 32-col stationary [V16|A1] x 64-col moving B
       accumulated into PSUM. Columns are processed in quads with
       tile_position=(0,32j) column tiling so each matmul writes its
       own 32-partition PSUM block and the four 32-col LDWEIGHTS
       interleave with matmuls of other column strips.
PSUM [128, 64]: block j rows [32j+a] = sse partials (a<16) / counts
(rows 32j+16+a) for column-residue j. Host sums the 4 blocks + 8 cores.
One-hots are exact in bf16; PSUM accumulates fp32; err2 rounded to bf16
(final loss rel err ~1e-4).

Measured (paired repeat-1-vs-9 wall-clock slope, 8 cores): ~540-590 us
across runs (rel err 4.5e-5) vs the session-start baseline's ~1.72 ms
under the same measurement (1123197 ns under its original noisier
method).
"""
import numpy as np
import ml_dtypes

import bass_rust
import concourse.bass as bass
import concourse.mybir as mybir
from concourse import tile as tile_mod
from concourse.tile import TileContext
from concourse.vector_clock import ScopedClock

F32 = mybir.dt.float32
BF16 = mybir.dt.bfloat16
I32 = mybir.dt.int32
I16 = mybir.dt.int16

N = 16_777_216
S = 1024
EPS = 1e-6
N_CORES = 8
P = 128
F = N // N_CORES // P          # 16384 free-dim elements per partition
WA, WB = 16, 64                # s = hi*WB + lo
G = 512                        # columns per pipelined chunk (32*512 = 16384)
CHUNKS = [(i * G, G) for i in range(F // G)] + (
    [(F - F % G, F % G)] if F % G else [])


# --- workarounds for this toolchain's walrus: it rejects >1 sync wait per
# --- instruction (setupSyncWait), including on Drain/NoOp (TPB_CTRL).

def _drain_and_barrier(self, tick_clock, wait_clock):
    nop0 = self.nc.sync.nop(nofuse=True)
    wait_clock.add_sem_waits(nop0.ins, ScopedClock({None: tick_clock.global_clock}))
    si = nop0.ins.sync_info
    waits = list(si.on_wait) if si is not None else []
    if len(waits) > 1:
        nop0.ins.sync_info = bass_rust.SyncInfo(on_wait=[waits[0]], on_update=[])
        for w in waits[1:]:
            nop = self.nc.sync.nop(nofuse=True)
            nop.ins.sync_info = bass_rust.SyncInfo(on_wait=[w], on_update=[])
    self.nc.sync.drain()
    self.nc.all_engine_barrier()
    popped = self.nc._tile_sem_poison_stack.pop()
    assert popped is self._sem_poison
    self.nc.clear_and_free_semaphores(list(self.sems.allocated().values()))
    self.nc.all_engine_barrier()


tile_mod.TileContext._drain_and_barrier = _drain_and_barrier


def _split_multi_waits(nc, max_waits=1):
    n = 0
    for f in nc.m.functions:
        for b in f.blocks:
            out, changed = [], False
            for i in b.instructions:
                si = i.sync_info
                waits = list(si.on_wait) if si is not None else []
                if len(waits) > max_waits:
                    for w in waits[:-max_waits]:
                        nop = bass_rust.InstNoOp(name=f"W-split-{n}")
                        n += 1
                        nop.engine = i.engine
                        nop.sync_info = bass_rust.SyncInfo(on_wait=[w], on_update=[])
                        out.append(nop)
                    i.sync_info = bass_rust.SyncInfo(
                        on_wait=waits[-max_waits:], on_update=list(si.on_update))
                    changed = True
                out.append(i)
            if changed:
                b.instructions = out


ACT_A_CLASSES = list(range(7, WA))   # A-classes built on the Scalar engine


def build_program(repeat=1):
    """Build the per-core Bass program (SPMD: same program, 8 data shards)."""
    n_chunks = len(CHUNKS)
    nt = n_chunks * repeat
    nc = bass.Bass()
    for a in ACT_A_CLASSES:
        t = nc.alloc_sbuf_tensor(f"const-f32-neg{a}", [128, 1], F32)
        nc.gpsimd.memset(t.ap(), float(-WB * a))
        nc.const_aps.aps[(F32, float(-WB * a))] = t.ap()
    nc.all_engine_barrier()
    yp = nc.declare_dram_parameter("yp", [P, F], BF16, isOutput=False)
    yt = nc.declare_dram_parameter("yt", [P, F], BF16, isOutput=False)
    st = nc.declare_dram_parameter("st", [P, F], I16, isOutput=False)
    out = nc.declare_dram_parameter("out", [P, WB], F32, isOutput=True)

    with TileContext(nc) as tc:
        with (
            tc.tile_pool(name="inp", bufs=2) as pin,
            tc.tile_pool(name="mask", bufs=2) as pm,
            tc.tile_pool(name="acc", bufs=1, space="PSUM") as pp,
            tc.tile_pool(name="res", bufs=1) as pr,
        ):
            psum = pp.tile([P, WB], F32)
            for t in range(nt):
                tt = t % n_chunks
                c0, Gc = CHUNKS[tt]
                sl = slice(c0, c0 + Gc)
                ypt = pin.tile([P, Gc], BF16, tag="yp")
                nc.sync.dma_start(out=ypt[:], in_=yp[:, sl])
                ytt = pin.tile([P, Gc], BF16, tag="yt")
                nc.sync.dma_start(out=ytt[:], in_=yt[:, sl])
                stt = pin.tile([P, Gc], I16, tag="st")
                nc.sync.dma_start(out=stt[:], in_=st[:, sl])

                lo_i = pin.tile([P, Gc], I16, tag="lo_i")
                nc.vector.tensor_scalar(lo_i[:], stt[:], WB - 1, None,
                                        mybir.AluOpType.bitwise_and)
                hi = pin.tile([P, Gc], BF16, tag="hi")
                nc.vector.tensor_sub(hi[:], stt[:], lo_i[:])
                scratch = pin.tile([P, Gc], BF16, tag="scratch")
                nc.vector.tensor_sub(scratch[:], ypt[:], ytt[:])
                err2 = pin.tile([P, Gc], BF16, tag="err2")
                nc.scalar.activation(err2[:], scratch[:],
                                     mybir.ActivationFunctionType.Square)

                # stationary: cols 0..15 = V16 (err2 masked by hi), 16..31 = A1
                aa = pm.tile([P, 2 * WA, Gc], BF16, tag="aa")
                sq = scratch
                for a in range(WA):
                    if a in ACT_A_CLASSES:
                        # exact one-hot on ACT: relu(1 - (hi - 64a)^2);
                        # hi holds the unshifted high bits (st & 960)
                        nc.scalar.activation(sq[:], hi[:],
                                             mybir.ActivationFunctionType.Square,
                                             bias=float(-WB * a))
                        nc.scalar.activation(aa[:, WA + a, :], sq[:],
                                             mybir.ActivationFunctionType.Relu,
                                             bias=1.0, scale=-1.0)
                    else:
                        nc.vector.tensor_scalar(aa[:, WA + a, :], hi[:],
                                                float(WB * a), None,
                                                mybir.AluOpType.is_equal)
                e_bc = err2[:].unsqueeze(1).broadcast_to([P, WA, Gc])
                nc.vector.tensor_tensor(aa[:, 0:WA, :], aa[:, WA:2 * WA, :],
                                        e_bc, mybir.AluOpType.mult)

                # moving: B[p, j, :] = (lo == j)
                bm = pm.tile([P, WB, Gc], BF16, tag="bm")
                for j in range(WB):
                    nc.vector.tensor_scalar(bm[:, j, :], lo_i[:], float(j),
                                            None, mybir.AluOpType.is_equal)

                for q in range(Gc // 4):
                    for j in range(4):
                        c = 4 * q + j
                        nc.tensor.matmul(
                            psum[32 * j:32 * j + 32, :],
                            aa[:, :, c], bm[:, :, c],
                            start=(t == 0 and q == 0),
                            stop=(t == nt - 1 and q == Gc // 4 - 1),
                            tile_position=(0, 32 * j))

            res = pr.tile([P, WB], F32)
            nc.scalar.copy(res[:], psum[:])
            nc.sync.dma_start(out=out[:], in_=res[:])
    _split_multi_waits(nc)
    return nc


# --- PJRT runner (axon path): jitted shard_map over 8 cores with
# --- device-resident inputs, reusable across calls.

def _make_runner(nc, n_cores=N_CORES):
    import jax
    from jax.sharding import Mesh, PartitionSpec, NamedSharding
    from jax.experimental.shard_map import shard_map
    from concourse.bass2jax import (_bass_exec_p, install_neuronx_cc_hook,
                                    partition_id_tensor)

    install_neuronx_cc_hook()
    partition_name = nc.partition_id_tensor.name if nc.partition_id_tensor else None
    in_names, out_names, out_avals, zero_outs = [], [], [], []
    for alloc in nc.m.functions[0].allocations:
        if not isinstance(alloc, mybir.MemoryLocationSet):
            continue
        name = alloc.memorylocations[0].name
        if alloc.kind == "ExternalInput":
            if name != partition_name:
                in_names.append(name)
        elif alloc.kind == "ExternalOutput":
            out_names.append(name)
            shape = tuple(alloc.tensor_shape)
            dtype = mybir.dt.np(alloc.dtype)
            out_avals.append(jax.core.ShapedArray(shape, dtype))
            zero_outs.append(np.zeros(shape, dtype))
    n_params = len(in_names)
    n_outs = len(out_avals)
    all_in_names = list(in_names) + list(out_names)
    if partition_name is not None:
        all_in_names.append(partition_name)

    def _body(*args):
        operands = list(args)
        if partition_name is not None:
            operands.append(partition_id_tensor())
        return tuple(_bass_exec_p.bind(
            *operands,
            out_avals=tuple(out_avals),
            in_names=tuple(all_in_names),
            out_names=tuple(out_names),
            lowering_input_output_aliases=(),
            sim_require_finite=True,
            sim_require_nnan=True,
            nc=nc,
        ))

    devices = jax.devices()[:n_cores]
    mesh = Mesh(np.asarray(devices), ("core",))
    sharded = jax.jit(
        shard_map(_body, mesh=mesh,
                  in_specs=(PartitionSpec("core"),) * (n_params + n_outs),
                  out_specs=(PartitionSpec("core"),) * n_outs,
                  check_rep=False),
        keep_unused=True,
    )
    sh = NamedSharding(mesh, PartitionSpec("core"))

    class Runner:
        def put_inputs(self, in_maps):
            concat = [np.concatenate([np.asarray(m[n]) for m in in_maps], axis=0)
                      for n in in_names]
            self.dev_in = [jax.device_put(a, sh) for a in concat]
            self.zeros = [jax.device_put(
                np.zeros((n_cores * z.shape[0], *z.shape[1:]), z.dtype), sh)
                for z in zero_outs]
            jax.block_until_ready(self.dev_in)
            jax.block_until_ready(self.zeros)

        def run(self):
            outs = sharded(*self.dev_in, *self.zeros)
            jax.block_until_ready(outs)
            return outs

        def results(self, outs):
            return [{n: np.asarray(outs[i]).reshape(n_cores, *out_avals[i].shape)[c]
                     for i, n in enumerate(out_names)} for c in range(n_cores)]

    return Runner()


_RUNNER_CACHE = {}


def get_runner(repeat=1):
    key = repeat
    if key not in _RUNNER_CACHE:
        _RUNNER_CACHE[key] = _make_runner(build_program(repeat=repeat))
    return _RUNNER_CACHE[key]


def shard_inputs(y_pred, y_true, stations):
    ypc = np.asarray(y_pred, dtype=np.float32).astype(ml_dtypes.bfloat16).reshape(N_CORES, P, F)
    ytc = np.asarray(y_true, dtype=np.float32).astype(ml_dtypes.bfloat16).reshape(N_CORES, P, F)
    stc = np.ascontiguousarray(stations, dtype=np.int32).astype(np.int16).reshape(N_CORES, P, F)
    return [{"yp": ypc[c], "yt": ytc[c], "st": stc[c]}
            for c in range(N_CORES)]


def finish_host(partials, station_std):
    """Sum the 8 cores' [128, 64] psum tiles: rows 32j+a (a<16) hold sse
    partials for column-residue j, rows 32j+16+a hold counts. Station id
    s = a*64 + b (hi*64 + lo)."""
    tot = np.sum(np.stack(partials, 0), axis=0, dtype=np.float32)  # [128, 64]
    blocks = tot.reshape(4, 32, WB).sum(axis=0)                    # [32, 64]
    sse = blocks[0:WA, :].reshape(-1)          # index = a*64 + b = station id
    cnt = blocks[WA:2 * WA, :].reshape(-1)
    mse = sse / np.maximum(cnt, np.float32(1.0))
    denom = (station_std.astype(np.float32) + np.float32(EPS)) ** 2
    present = cnt > 0
    per_station = np.where(present, mse / denom, np.float32(0.0))
    n_present = np.maximum(np.sum(present.astype(np.float32)), np.float32(1.0))
    return (np.sum(per_station) / n_present).astype(np.float32)


def kernel(y_pred, y_true, stations, station_std):
    runner = get_runner()
    runner.put_inputs(shard_inputs(y_pred, y_true, stations))
    outs = runner.run()
    res = runner.results(outs)
    partials = [res[c]["out"] for c in range(N_CORES)]
    loss = finish_host(partials, np.asarray(station_std))
    return np.asarray(loss, dtype=np.float32)


# revision 27
# speedup vs baseline: 2.1457x; 2.1457x over previous
"""Trainium2 Bass kernel for nn_NSELoss (segment-reduce NSE loss).

Contract: kernel(**inputs) takes the FULL inputs
  y_pred [16777216] f32, y_true [16777216] f32,
  stations [16777216] i32, station_std [1024] f32
and returns the full scalar output (f32), matching reference().
y_pred/y_true are cast to bf16 and stations to int16 during host-side
sharding (cuts HBM traffic 24 -> 12 MB/core; the on-device math is bf16
anyway and station ids < 1024 are exact in i16 — adds ~1e-4-level noise
against the 2e-2 tolerance). The hi digit is kept unshifted (st & 960)
and compared against 64*a, since i16 shifts fail walrus's ISA check.

Sharding: data-parallel over N across 8 NeuronCores. Each core computes
partial sse[1024] and cnt[1024]; host sums the 8 partials and finishes.

Device algorithm (per core, 2,097,152 elements as [128, 16384]):
Split the station id s = hi*64 + lo (hi 4 bits, lo 6 bits). Per chunk of
G columns:
  DVE: lo/hi extraction (tensor_scalar 2x + copy), diff = yp - yt,
       A1[a]   = (hi == a), a in 0..6    tensor_scalar is_equal @ 4x mode
       V16[16] = A1 * err2               ONE fused tensor_tensor (bcast
                                         on the outer AP dim) @ 2x mode
       B[64]   = (lo == j)               64x tensor_scalar is_equal @ 4x
  ACT: err2 = Square(diff) -> bf16, and A1[a] for a in 7..15 via the
       exact integer one-hot relu(1 - (hi - a)^2)  (2 passes/class) —
       offloads the Vector engine, which is otherwise the bottleneck.
  PE:  per column c, one 32-col stationary [V16|A1] x 64-col moving B
       accumulated into PSUM. Columns are processed in quads with
       tile_position=(0,32j) column tiling so each matmul writes its
       own 32-partition PSUM block and the four 32-col LDWEIGHTS
       interleave with matmuls of other column strips.
PSUM [128, 64]: block j rows [32j+a] = sse partials (a<16) / counts
(rows 32j+16+a) for column-residue j. Host sums the 4 blocks + 8 cores.
One-hots are exact in bf16; PSUM accumulates fp32; err2 rounded to bf16
(final loss rel err ~1e-4).

Measured (paired repeat-1-vs-9 wall-clock slope, 8 cores): ~540-590 us
across runs (rel err 4.5e-5) vs the session-start baseline's ~1.72 ms
under the same measurement (1123197 ns under its original noisier
method).
"""
import numpy as np
import ml_dtypes

import bass_rust
import concourse.bass as bass
import concourse.mybir as mybir
from concourse import tile as tile_mod
from concourse.tile import TileContext
from concourse.vector_clock import ScopedClock

F32 = mybir.dt.float32
BF16 = mybir.dt.bfloat16
I32 = mybir.dt.int32
I16 = mybir.dt.int16

N = 16_777_216
S = 1024
EPS = 1e-6
N_CORES = 8
P = 128
F = N // N_CORES // P          # 16384 free-dim elements per partition
WA, WB = 16, 64                # s = hi*WB + lo
G = 448                        # columns per pipelined chunk (36*448 + 256 = 16384)
CHUNKS = [(i * G, G) for i in range(F // G)] + (
    [(F - F % G, F % G)] if F % G else [])


# --- workarounds for this toolchain's walrus: it rejects >1 sync wait per
# --- instruction (setupSyncWait), including on Drain/NoOp (TPB_CTRL).

def _drain_and_barrier(self, tick_clock, wait_clock):
    nop0 = self.nc.sync.nop(nofuse=True)
    wait_clock.add_sem_waits(nop0.ins, ScopedClock({None: tick_clock.global_clock}))
    si = nop0.ins.sync_info
    waits = list(si.on_wait) if si is not None else []
    if len(waits) > 1:
        nop0.ins.sync_info = bass_rust.SyncInfo(on_wait=[waits[0]], on_update=[])
        for w in waits[1:]:
            nop = self.nc.sync.nop(nofuse=True)
            nop.ins.sync_info = bass_rust.SyncInfo(on_wait=[w], on_update=[])
    self.nc.sync.drain()
    self.nc.all_engine_barrier()
    popped = self.nc._tile_sem_poison_stack.pop()
    assert popped is self._sem_poison
    self.nc.clear_and_free_semaphores(list(self.sems.allocated().values()))
    self.nc.all_engine_barrier()


tile_mod.TileContext._drain_and_barrier = _drain_and_barrier


def _split_multi_waits(nc, max_waits=1):
    n = 0
    for f in nc.m.functions:
        for b in f.blocks:
            out, changed = [], False
            for i in b.instructions:
                si = i.sync_info
                waits = list(si.on_wait) if si is not None else []
                if len(waits) > max_waits:
                    for w in waits[:-max_waits]:
                        nop = bass_rust.InstNoOp(name=f"W-split-{n}")
                        n += 1
                        nop.engine = i.engine
                        nop.sync_info = bass_rust.SyncInfo(on_wait=[w], on_update=[])
                        out.append(nop)
                    i.sync_info = bass_rust.SyncInfo(
                        on_wait=waits[-max_waits:], on_update=list(si.on_update))
                    changed = True
                out.append(i)
            if changed:
                b.instructions = out


ACT_A_CLASSES = list(range(7, WA))   # A-classes built on the Scalar engine


def build_program(repeat=1):
    """Build the per-core Bass program (SPMD: same program, 8 data shards)."""
    n_chunks = len(CHUNKS)
    nt = n_chunks * repeat
    nc = bass.Bass()
    for a in ACT_A_CLASSES:
        t = nc.alloc_sbuf_tensor(f"const-f32-neg{a}", [128, 1], F32)
        nc.gpsimd.memset(t.ap(), float(-WB * a))
        nc.const_aps.aps[(F32, float(-WB * a))] = t.ap()
    nc.all_engine_barrier()
    yp = nc.declare_dram_parameter("yp", [P, F], BF16, isOutput=False)
    yt = nc.declare_dram_parameter("yt", [P, F], BF16, isOutput=False)
    st = nc.declare_dram_parameter("st", [P, F], I16, isOutput=False)
    out = nc.declare_dram_parameter("out", [P, WB], F32, isOutput=True)

    with TileContext(nc) as tc:
        with (
            tc.tile_pool(name="inp", bufs=2) as pin,
            tc.tile_pool(name="mask", bufs=2) as pm,
            tc.tile_pool(name="acc", bufs=1, space="PSUM") as pp,
            tc.tile_pool(name="res", bufs=1) as pr,
        ):
            psum = pp.tile([P, WB], F32)
            for t in range(nt):
                tt = t % n_chunks
                c0, Gc = CHUNKS[tt]
                sl = slice(c0, c0 + Gc)
                ypt = pin.tile([P, Gc], BF16, tag="yp")
                nc.sync.dma_start(out=ypt[:], in_=yp[:, sl])
                ytt = pin.tile([P, Gc], BF16, tag="yt")
                nc.sync.dma_start(out=ytt[:], in_=yt[:, sl])
                stt = pin.tile([P, Gc], I16, tag="st")
                nc.sync.dma_start(out=stt[:], in_=st[:, sl])

                lo_i = pin.tile([P, Gc], I16, tag="lo_i")
                nc.vector.tensor_scalar(lo_i[:], stt[:], WB - 1, None,
                                        mybir.AluOpType.bitwise_and)
                lo = pin.tile([P, Gc], BF16, tag="lo")
                nc.vector.tensor_copy(lo[:], lo_i[:])
                hi_i = pin.tile([P, Gc], I16, tag="hi_i")
                nc.vector.tensor_scalar(hi_i[:], stt[:], (WA - 1) * WB, None,
                                        mybir.AluOpType.bitwise_and)
                hi = pin.tile([P, Gc], BF16, tag="hi")
                nc.vector.tensor_copy(hi[:], hi_i[:])
                diff = pin.tile([P, Gc], BF16, tag="diff")
                nc.vector.tensor_sub(diff[:], ypt[:], ytt[:])
                err2 = pin.tile([P, Gc], BF16, tag="err2")
                nc.scalar.activation(err2[:], diff[:],
                                     mybir.ActivationFunctionType.Square)

                # stationary: cols 0..15 = V16 (err2 masked by hi), 16..31 = A1
                aa = pm.tile([P, 2 * WA, Gc], BF16, tag="aa")
                sq = pin.tile([P, Gc], BF16, tag="sq")
                for a in range(WA):
                    if a in ACT_A_CLASSES:
                        # exact one-hot on ACT: relu(1 - (hi - 64a)^2);
                        # hi holds the unshifted high bits (st & 960)
                        nc.scalar.activation(sq[:], hi[:],
                                             mybir.ActivationFunctionType.Square,
                                             bias=float(-WB * a))
                        nc.scalar.activation(aa[:, WA + a, :], sq[:],
                                             mybir.ActivationFunctionType.Relu,
                                             bias=1.0, scale=-1.0)
                    else:
                        nc.vector.tensor_scalar(aa[:, WA + a, :], hi[:],
                                                float(WB * a), None,
                                                mybir.AluOpType.is_equal)
                e_bc = err2[:].unsqueeze(1).broadcast_to([P, WA, Gc])
                nc.vector.tensor_tensor(aa[:, 0:WA, :], aa[:, WA:2 * WA, :],
                                        e_bc, mybir.AluOpType.mult)

                # moving: B[p, j, :] = (lo == j)
                bm = pm.tile([P, WB, Gc], BF16, tag="bm")
                for j in range(WB):
                    nc.vector.tensor_scalar(bm[:, j, :], lo[:], float(j),
                                            None, mybir.AluOpType.is_equal)

                for q in range(Gc // 4):
                    for j in range(4):
                        c = 4 * q + j
                        nc.tensor.matmul(
                            psum[32 * j:32 * j + 32, :],
                            aa[:, :, c], bm[:, :, c],
                            start=(t == 0 and q == 0),
                            stop=(t == nt - 1 and q == Gc // 4 - 1),
                            tile_position=(0, 32 * j))

            res = pr.tile([P, WB], F32)
            nc.scalar.copy(res[:], psum[:])
            nc.sync.dma_start(out=out[:], in_=res[:])
    _split_multi_waits(nc)
    return nc


# --- PJRT runner (axon path): jitted shard_map over 8 cores with
# --- device-resident inputs, reusable across calls.

def _make_runner(nc, n_cores=N_CORES):
    import jax
    from jax.sharding import Mesh, PartitionSpec, NamedSharding
    from jax.experimental.shard_map import shard_map
    from concourse.bass2jax import (_bass_exec_p, install_neuronx_cc_hook,
                                    partition_id_tensor)

    install_neuronx_cc_hook()
    partition_name = nc.partition_id_tensor.name if nc.partition_id_tensor else None
    in_names, out_names, out_avals, zero_outs = [], [], [], []
    for alloc in nc.m.functions[0].allocations:
        if not isinstance(alloc, mybir.MemoryLocationSet):
            continue
        name = alloc.memorylocations[0].name
        if alloc.kind == "ExternalInput":
            if name != partition_name:
                in_names.append(name)
        elif alloc.kind == "ExternalOutput":
            out_names.append(name)
            shape = tuple(alloc.tensor_shape)
            dtype = mybir.dt.np(alloc.dtype)
            out_avals.append(jax.core.ShapedArray(shape, dtype))
            zero_outs.append(np.zeros(shape, dtype))
    n_params = len(in_names)
    n_outs = len(out_avals)
    all_in_names = list(in_names) + list(out_names)
    if partition_name is not None:
        all_in_names.append(partition_name)

    def _body(*args):
        operands = list(args)
        if partition_name is not None:
            operands.append(partition_id_tensor())
        return tuple(_bass_exec_p.bind(
            *operands,
            out_avals=tuple(out_avals),
            in_names=tuple(all_in_names),
            out_names=tuple(out_names),
            lowering_input_output_aliases=(),
            sim_require_finite=True,
            sim_require_nnan=True,
            nc=nc,
        ))

    devices = jax.devices()[:n_cores]
    mesh = Mesh(np.asarray(devices), ("core",))
    sharded = jax.jit(
        shard_map(_body, mesh=mesh,
                  in_specs=(PartitionSpec("core"),) * (n_params + n_outs),
                  out_specs=(PartitionSpec("core"),) * n_outs,
                  check_rep=False),
        keep_unused=True,
    )
    sh = NamedSharding(mesh, PartitionSpec("core"))

    class Runner:
        def put_inputs(self, in_maps):
            concat = [np.concatenate([np.asarray(m[n]) for m in in_maps], axis=0)
                      for n in in_names]
            self.dev_in = [jax.device_put(a, sh) for a in concat]
            self.zeros = [jax.device_put(
                np.zeros((n_cores * z.shape[0], *z.shape[1:]), z.dtype), sh)
                for z in zero_outs]
            jax.block_until_ready(self.dev_in)
            jax.block_until_ready(self.zeros)

        def run(self):
            outs = sharded(*self.dev_in, *self.zeros)
            jax.block_until_ready(outs)
            return outs

        def results(self, outs):
            return [{n: np.asarray(outs[i]).reshape(n_cores, *out_avals[i].shape)[c]
                     for i, n in enumerate(out_names)} for c in range(n_cores)]

    return Runner()


_RUNNER_CACHE = {}


def get_runner(repeat=1):
    key = repeat
    if key not in _RUNNER_CACHE:
        _RUNNER_CACHE[key] = _make_runner(build_program(repeat=repeat))
    return _RUNNER_CACHE[key]


def shard_inputs(y_pred, y_true, stations):
    ypc = np.asarray(y_pred, dtype=np.float32).astype(ml_dtypes.bfloat16).reshape(N_CORES, P, F)
    ytc = np.asarray(y_true, dtype=np.float32).astype(ml_dtypes.bfloat16).reshape(N_CORES, P, F)
    stc = np.ascontiguousarray(stations, dtype=np.int32).astype(np.int16).reshape(N_CORES, P, F)
    return [{"yp": ypc[c], "yt": ytc[c], "st": stc[c]}
            for c in range(N_CORES)]


def finish_host(partials, station_std):
    """Sum the 8 cores' [128, 64] psum tiles: rows 32j+a (a<16) hold sse
    partials for column-residue j, rows 32j+16+a hold counts. Station id
    s = a*64 + b (hi*64 + lo)."""
    tot = np.sum(np.stack(partials, 0), axis=0, dtype=np.float32)  # [128, 64]
    blocks = tot.reshape(4, 32, WB).sum(axis=0)                    # [32, 64]
    sse = blocks[0:WA, :].reshape(-1)          # index = a*64 + b = station id
    cnt = blocks[WA:2 * WA, :].reshape(-1)
    mse = sse / np.maximum(cnt, np.float32(1.0))
    denom = (station_std.astype(np.float32) + np.float32(EPS)) ** 2
    present = cnt > 0
    per_station = np.where(present, mse / denom, np.float32(0.0))
    n_present = np.maximum(np.sum(present.astype(np.float32)), np.float32(1.0))
    return (np.sum(per_station) / n_present).astype(np.float32)


def kernel(y_pred, y_true, stations, station_std):
    runner = get_runner()
    runner.put_inputs(shard_inputs(y_pred, y_true, stations))
    outs = runner.run()
    res = runner.results(outs)
    partials = [res[c]["out"] for c in range(N_CORES)]
    loss = finish_host(partials, np.asarray(station_std))
    return np.asarray(loss, dtype=np.float32)


# revision 29
# speedup vs baseline: 2.6674x; 1.2431x over previous
"""Trainium2 Bass kernel for nn_NSELoss (segment-reduce NSE loss).

Contract: kernel(**inputs) takes the FULL inputs
  y_pred [16777216] f32, y_true [16777216] f32,
  stations [16777216] i32, station_std [1024] f32
and returns the full scalar output (f32), matching reference().
y_pred/y_true are cast to bf16 and stations to int16 during host-side
sharding (cuts HBM traffic 24 -> 12 MB/core; the on-device math is bf16
anyway and station ids < 1024 are exact in i16 — adds ~1e-4-level noise
against the 2e-2 tolerance). The hi digit is kept unshifted (st & 960)
and compared against 64*a, since i16 shifts fail walrus's ISA check.

Sharding: data-parallel over N across 8 NeuronCores. Each core computes
partial sse[1024] and cnt[1024]; host sums the 8 partials and finishes.

Device algorithm (per core, 2,097,152 elements as [128, 16384]):
Split the station id s = hi*64 + lo (hi 4 bits, lo 6 bits). Per chunk of
G columns:
  DVE: lo/hi extraction (tensor_scalar 2x + copy), diff = yp - yt,
       A1[a]   = (hi == a), a in 0..6    tensor_scalar is_equal @ 4x mode
       V16[16] = A1 * err2               ONE fused tensor_tensor (bcast
                                         on the outer AP dim) @ 2x mode
       B[64]   = (lo == j)               64x tensor_scalar is_equal @ 4x
  ACT: err2 = Square(diff) -> bf16, and A1[a] for a in 7..15 via the
       exact integer one-hot relu(1 - (hi - a)^2)  (2 passes/class) —
       offloads the Vector engine, which is otherwise the bottleneck.
  PE:  per column c, one 32-col stationary [V16|A1] x 64-col moving B
       accumulated into PSUM. Columns are processed in quads with
       tile_position=(0,32j) column tiling so each matmul writes its
       own 32-partition PSUM block and the four 32-col LDWEIGHTS
       interleave with matmuls of other column strips.
PSUM [128, 64]: block j rows [32j+a] = sse partials (a<16) / counts
(rows 32j+16+a) for column-residue j. Host sums the 4 blocks + 8 cores.
One-hots are exact in bf16; PSUM accumulates fp32; err2 rounded to bf16
(final loss rel err ~1e-4).

Measured (paired repeat-1-vs-9 wall-clock slope, 8 cores): ~540-590 us
across runs (rel err 4.5e-5) vs the session-start baseline's ~1.72 ms
under the same measurement (1123197 ns under its original noisier
method).
"""
import numpy as np
import ml_dtypes

import bass_rust
import concourse.bass as bass
import concourse.mybir as mybir
from concourse import tile as tile_mod
from concourse.tile import TileContext
from concourse.vector_clock import ScopedClock

F32 = mybir.dt.float32
BF16 = mybir.dt.bfloat16
I32 = mybir.dt.int32
I16 = mybir.dt.int16

N = 16_777_216
S = 1024
EPS = 1e-6
N_CORES = 8
P = 128
F = N // N_CORES // P          # 16384 free-dim elements per partition
WA, WB = 16, 64                # s = hi*WB + lo
G = 504                        # columns per pipelined chunk (32*504 + 256 = 16384)
CHUNKS = [(i * G, G) for i in range(F // G)] + (
    [(F - F % G, F % G)] if F % G else [])


# --- workarounds for this toolchain's walrus: it rejects >1 sync wait per
# --- instruction (setupSyncWait), including on Drain/NoOp (TPB_CTRL).

def _drain_and_barrier(self, tick_clock, wait_clock):
    nop0 = self.nc.sync.nop(nofuse=True)
    wait_clock.add_sem_waits(nop0.ins, ScopedClock({None: tick_clock.global_clock}))
    si = nop0.ins.sync_info
    waits = list(si.on_wait) if si is not None else []
    if len(waits) > 1:
        nop0.ins.sync_info = bass_rust.SyncInfo(on_wait=[waits[0]], on_update=[])
        for w in waits[1:]:
            nop = self.nc.sync.nop(nofuse=True)
            nop.ins.sync_info = bass_rust.SyncInfo(on_wait=[w], on_update=[])
    self.nc.sync.drain()
    self.nc.all_engine_barrier()
    popped = self.nc._tile_sem_poison_stack.pop()
    assert popped is self._sem_poison
    self.nc.clear_and_free_semaphores(list(self.sems.allocated().values()))
    self.nc.all_engine_barrier()


tile_mod.TileContext._drain_and_barrier = _drain_and_barrier


def _split_multi_waits(nc, max_waits=1):
    n = 0
    for f in nc.m.functions:
        for b in f.blocks:
            out, changed = [], False
            for i in b.instructions:
                si = i.sync_info
                waits = list(si.on_wait) if si is not None else []
                if len(waits) > max_waits:
                    for w in waits[:-max_waits]:
                        nop = bass_rust.InstNoOp(name=f"W-split-{n}")
                        n += 1
                        nop.engine = i.engine
                        nop.sync_info = bass_rust.SyncInfo(on_wait=[w], on_update=[])
                        out.append(nop)
                    i.sync_info = bass_rust.SyncInfo(
                        on_wait=waits[-max_waits:], on_update=list(si.on_update))
                    changed = True
                out.append(i)
            if changed:
                b.instructions = out


ACT_A_CLASSES = list(range(7, WA))   # A-classes built on the Scalar engine


def build_program(repeat=1):
    """Build the per-core Bass program (SPMD: same program, 8 data shards)."""
    n_chunks = len(CHUNKS)
    nt = n_chunks * repeat
    nc = bass.Bass()
    for a in ACT_A_CLASSES:
        t = nc.alloc_sbuf_tensor(f"const-f32-neg{a}", [128, 1], F32)
        nc.gpsimd.memset(t.ap(), float(-WB * a))
        nc.const_aps.aps[(F32, float(-WB * a))] = t.ap()
    nc.all_engine_barrier()
    yp = nc.declare_dram_parameter("yp", [P, F], BF16, isOutput=False)
    yt = nc.declare_dram_parameter("yt", [P, F], BF16, isOutput=False)
    st = nc.declare_dram_parameter("st", [P, F], I16, isOutput=False)
    out = nc.declare_dram_parameter("out", [P, WB], F32, isOutput=True)

    with TileContext(nc) as tc:
        with (
            tc.tile_pool(name="inp", bufs=2) as pin,
            tc.tile_pool(name="mask", bufs=2) as pm,
            tc.tile_pool(name="acc", bufs=1, space="PSUM") as pp,
            tc.tile_pool(name="res", bufs=1) as pr,
        ):
            psum = pp.tile([P, WB], F32)
            for t in range(nt):
                tt = t % n_chunks
                c0, Gc = CHUNKS[tt]
                sl = slice(c0, c0 + Gc)
                ypt = pin.tile([P, Gc], BF16, tag="yp")
                nc.sync.dma_start(out=ypt[:], in_=yp[:, sl])
                ytt = pin.tile([P, Gc], BF16, tag="yt")
                nc.sync.dma_start(out=ytt[:], in_=yt[:, sl])
                stt = pin.tile([P, Gc], I16, tag="st")
                nc.sync.dma_start(out=stt[:], in_=st[:, sl])

                lo_i = pin.tile([P, Gc], I16, tag="lo_i")
                nc.vector.tensor_scalar(lo_i[:], stt[:], WB - 1, None,
                                        mybir.AluOpType.bitwise_and)
                lo = pin.tile([P, Gc], BF16, tag="lo")
                nc.vector.tensor_copy(lo[:], lo_i[:])
                hi = pin.tile([P, Gc], BF16, tag="hi")
                nc.vector.tensor_sub(hi[:], stt[:], lo_i[:])
                scratch = pin.tile([P, Gc], BF16, tag="scratch")
                nc.vector.tensor_sub(scratch[:], ypt[:], ytt[:])
                err2 = pin.tile([P, Gc], BF16, tag="err2")
                nc.scalar.activation(err2[:], scratch[:],
                                     mybir.ActivationFunctionType.Square)

                # stationary: cols 0..15 = V16 (err2 masked by hi), 16..31 = A1
                aa = pm.tile([P, 2 * WA, Gc], BF16, tag="aa")
                sq = scratch
                for a in range(WA):
                    if a in ACT_A_CLASSES:
                        # exact one-hot on ACT: relu(1 - (hi - 64a)^2);
                        # hi holds the unshifted high bits (st & 960)
                        nc.scalar.activation(sq[:], hi[:],
                                             mybir.ActivationFunctionType.Square,
                                             bias=float(-WB * a))
                        nc.scalar.activation(aa[:, WA + a, :], sq[:],
                                             mybir.ActivationFunctionType.Relu,
                                             bias=1.0, scale=-1.0)
                    else:
                        nc.vector.tensor_scalar(aa[:, WA + a, :], hi[:],
                                                float(WB * a), None,
                                                mybir.AluOpType.is_equal)
                e_bc = err2[:].unsqueeze(1).broadcast_to([P, WA, Gc])
                nc.vector.tensor_tensor(aa[:, 0:WA, :], aa[:, WA:2 * WA, :],
                                        e_bc, mybir.AluOpType.mult)

                # moving: B[p, j, :] = (lo == j)
                bm = pm.tile([P, WB, Gc], BF16, tag="bm")
                for j in range(WB):
                    nc.vector.tensor_scalar(bm[:, j, :], lo[:], float(j),
                                            None, mybir.AluOpType.is_equal)

                for q in range(Gc // 4):
                    for j in range(4):
                        c = 4 * q + j
                        nc.tensor.matmul(
                            psum[32 * j:32 * j + 32, :],
                            aa[:, :, c], bm[:, :, c],
                            start=(t == 0 and q == 0),
                            stop=(t == nt - 1 and q == Gc // 4 - 1),
                            tile_position=(0, 32 * j))

            res = pr.tile([P, WB], F32)
            nc.scalar.copy(res[:], psum[:])
            nc.sync.dma_start(out=out[:], in_=res[:])
    _split_multi_waits(nc)
    return nc


# --- PJRT runner (axon path): jitted shard_map over 8 cores with
# --- device-resident inputs, reusable across calls.

def _make_runner(nc, n_cores=N_CORES):
    import jax
    from jax.sharding import Mesh, PartitionSpec, NamedSharding
    from jax.experimental.shard_map import shard_map
    from concourse.bass2jax import (_bass_exec_p, install_neuronx_cc_hook,
                                    partition_id_tensor)

    install_neuronx_cc_hook()
    partition_name = nc.partition_id_tensor.name if nc.partition_id_tensor else None
    in_names, out_names, out_avals, zero_outs = [], [], [], []
    for alloc in nc.m.functions[0].allocations:
        if not isinstance(alloc, mybir.MemoryLocationSet):
            continue
        name = alloc.memorylocations[0].name
        if alloc.kind == "ExternalInput":
            if name != partition_name:
                in_names.append(name)
        elif alloc.kind == "ExternalOutput":
            out_names.append(name)
            shape = tuple(alloc.tensor_shape)
            dtype = mybir.dt.np(alloc.dtype)
            out_avals.append(jax.core.ShapedArray(shape, dtype))
            zero_outs.append(np.zeros(shape, dtype))
    n_params = len(in_names)
    n_outs = len(out_avals)
    all_in_names = list(in_names) + list(out_names)
    if partition_name is not None:
        all_in_names.append(partition_name)

    def _body(*args):
        operands = list(args)
        if partition_name is not None:
            operands.append(partition_id_tensor())
        return tuple(_bass_exec_p.bind(
            *operands,
            out_avals=tuple(out_avals),
            in_names=tuple(all_in_names),
            out_names=tuple(out_names),
            lowering_input_output_aliases=(),
            sim_require_finite=True,
            sim_require_nnan=True,
            nc=nc,
        ))

    devices = jax.devices()[:n_cores]
    mesh = Mesh(np.asarray(devices), ("core",))
    sharded = jax.jit(
        shard_map(_body, mesh=mesh,
                  in_specs=(PartitionSpec("core"),) * (n_params + n_outs),
                  out_specs=(PartitionSpec("core"),) * n_outs,
                  check_rep=False),
        keep_unused=True,
    )
    sh = NamedSharding(mesh, PartitionSpec("core"))

    class Runner:
        def put_inputs(self, in_maps):
            concat = [np.concatenate([np.asarray(m[n]) for m in in_maps], axis=0)
                      for n in in_names]
            self.dev_in = [jax.device_put(a, sh) for a in concat]
            self.zeros = [jax.device_put(
                np.zeros((n_cores * z.shape[0], *z.shape[1:]), z.dtype), sh)
                for z in zero_outs]
            jax.block_until_ready(self.dev_in)
            jax.block_until_ready(self.zeros)

        def run(self):
            outs = sharded(*self.dev_in, *self.zeros)
            jax.block_until_ready(outs)
            return outs

        def results(self, outs):
            return [{n: np.asarray(outs[i]).reshape(n_cores, *out_avals[i].shape)[c]
                     for i, n in enumerate(out_names)} for c in range(n_cores)]

    return Runner()


_RUNNER_CACHE = {}


def get_runner(repeat=1):
    key = repeat
    if key not in _RUNNER_CACHE:
        _RUNNER_CACHE[key] = _make_runner(build_program(repeat=repeat))
    return _RUNNER_CACHE[key]


def shard_inputs(y_pred, y_true, stations):
    ypc = np.asarray(y_pred, dtype=np.float32).astype(ml_dtypes.bfloat16).reshape(N_CORES, P, F)
    ytc = np.asarray(y_true, dtype=np.float32).astype(ml_dtypes.bfloat16).reshape(N_CORES, P, F)
    stc = np.ascontiguousarray(stations, dtype=np.int32).astype(np.int16).reshape(N_CORES, P, F)
    return [{"yp": ypc[c], "yt": ytc[c], "st": stc[c]}
            for c in range(N_CORES)]


def finish_host(partials, station_std):
    """Sum the 8 cores' [128, 64] psum tiles: rows 32j+a (a<16) hold sse
    partials for column-residue j, rows 32j+16+a hold counts. Station id
    s = a*64 + b (hi*64 + lo)."""
    tot = np.sum(np.stack(partials, 0), axis=0, dtype=np.float32)  # [128, 64]
    blocks = tot.reshape(4, 32, WB).sum(axis=0)                    # [32, 64]
    sse = blocks[0:WA, :].reshape(-1)          # index = a*64 + b = station id
    cnt = blocks[WA:2 * WA, :].reshape(-1)
    mse = sse / np.maximum(cnt, np.float32(1.0))
    denom = (station_std.astype(np.float32) + np.float32(EPS)) ** 2
    present = cnt > 0
    per_station = np.where(present, mse / denom, np.float32(0.0))
    n_present = np.maximum(np.sum(present.astype(np.float32)), np.float32(1.0))
    return (np.sum(per_station) / n_present).astype(np.float32)


def kernel(y_pred, y_true, stations, station_std):
    runner = get_runner()
    runner.put_inputs(shard_inputs(y_pred, y_true, stations))
    outs = runner.run()
    res = runner.results(outs)
    partials = [res[c]["out"] for c in range(N_CORES)]
    loss = finish_host(partials, np.asarray(station_std))
    return np.asarray(loss, dtype=np.float32)
